# revision 21
# baseline (speedup 1.0000x reference)
"""BinaryTreeCRF inside algorithm on 8 Trainium2 NeuronCores.

Math per internal node p with children (left, right):
    inside[p] = em[p] + logsumexp_{l,r}( left[l] + right[r] + trans[p,l,r] )

Stable device formulation per level (n parent nodes):
    mxl[j] = max_l left[j,l];  mxr[j] = max_r right[j,r]
    B[j,(l,r)]   = (left[j,l]-mxl[j]) + (right[j,r]-mxr[j])       (PE matmuls)
    outer        = exp(B)                                          (ACT)
    S[j,p]       = sum_{lr} outer[j,lr] * exp(trans[p,l,r])        (PE matmuls)
    inside[j,p]  = em[j,p] + mxl[j] + mxr[j] + ln(S[j,p])          (ACT + DVE)

Sharding: core i owns the subtree over leaves [1024*i, 1024*(i+1)) and runs
10 levels (512,256,...,1 nodes) with zero communication (kernel A, SPMD x8).
The host concatenates the 8 subtree roots; kernel B (1 core) runs the top
3 levels (4,2,1 nodes). Complete binary tree => sibling pairs never cross
the 1024-leaf boundary, and within global level k core i's nodes are the
contiguous slice [i*m, (i+1)*m) of internal_emissions' level-k block.
"""

import os

import numpy as np

import concourse.bass as bass
import concourse.mybir as mybir
import concourse.tile as tile
from concourse import bass_utils

L = 32  # labels
N_LEAVES = 8192
N_CORES = 8
LPC = N_LEAVES // N_CORES  # leaves per core (1024)

F32 = mybir.dt.float32
BF16 = mybir.dt.bfloat16


def _level_sizes(n0):
    out = []
    n = n0
    while n > 1:
        n //= 2
        out.append(n)
    return out


def build_tree_nc(n0, num_devices):
    """Bass program: leaf scores [n0, 32] + emissions [n0-1, 32] -> root [1, 32]."""
    nc = bass.Bass("TRN2", target_bir_lowering=False, debug=False,
                   num_devices=num_devices)

    sizes = _level_sizes(n0)
    n_em = n0 - 1

    leaves_d = nc.dram_tensor("leaves", [n0, L], F32, kind="ExternalInput")
    em_d = nc.dram_tensor("emissions", [n_em, L], F32, kind="ExternalInput")
    # trans_lrp[(l*32+r), p] = trans[p, l, r]
    trans_d = nc.dram_tensor("trans_lrp", [L * L, L], F32, kind="ExternalInput")
    # wmat[k, c*128+m]: k<32: 1 if k == 4c + m//32 ; k>=32: 1 if (k-32) == m%32
    wmat_d = nc.dram_tensor("wmat", [2 * L, 1024], F32, kind="ExternalInput")
    ident_d = nc.dram_tensor("ident", [128, 128], F32, kind="ExternalInput")
    out_d = nc.dram_tensor("root_out", [1, L], F32, kind="ExternalOutput")

    with tile.TileContext(nc) as tc:
        with (
            tc.tile_pool(name="consts", bufs=1) as cpool,
            tc.tile_pool(name="scores", bufs=1) as spool,
            tc.tile_pool(name="work", bufs=2) as wpool,
            tc.tile_pool(name="psum", bufs=2, space="PSUM") as ppool,
        ):
            # ---- constants ----
            ident = cpool.tile([128, 128], F32)
            nc.gpsimd.dma_start(ident, ident_d.ap())
            ident_bf = cpool.tile([128, 128], BF16)
            nc.scalar.copy(ident_bf, ident)

            wmat_f = cpool.tile([2 * L, 1024], F32)
            nc.sync.dma_start(wmat_f, wmat_d.ap())
            wmat = cpool.tile([2 * L, 1024], BF16)
            nc.scalar.copy(wmat, wmat_f)

            texp_f = cpool.tile([128, 8 * L], F32)
            nc.gpsimd.dma_start(
                texp_f.rearrange("k (c p) -> k c p", c=8),
                trans_d.ap().rearrange("(c k) p -> k c p", k=128),
            )
            texp = cpool.tile([128, 8 * L], BF16)
            nc.scalar.activation(texp, texp_f, mybir.ActivationFunctionType.Exp)

            # ---- leaf scores ----
            t0 = max(1, n0 // 128)  # leaf tiles (chunks of <=128 nodes)
            p0 = min(n0, 128)
            cur = spool.tile([p0, t0 * L], F32, tag="lvl0", name="lvl0")
            if n0 >= 128:
                nc.sync.dma_start(
                    cur.rearrange("p (c m) -> p c m", c=t0),
                    leaves_d.ap().rearrange("(c p) m -> p c m", p=128),
                )
            else:
                nc.sync.dma_start(cur, leaves_d.ap())

            em_off = 0
            for li, n in enumerate(sizes):
                tk = max(1, n // 128)        # 128-node output chunks
                pk = min(n, 128)
                n_prev = 2 * n
                pchunks = max(1, n_prev // 128)
                m0 = min(n_prev, 128)

                # emissions for this level -> [pk, tk*L]
                if n >= 128:
                    em_t = wpool.tile([pk, tk * L], F32, tag="em", bufs=2,
                                      name=f"em{li}")
                    nc.sync.dma_start(
                        em_t.rearrange("p (c m) -> p c m", c=tk),
                        em_d.ap()[em_off:em_off + n, :]
                        .rearrange("(c p) m -> p c m", p=128),
                    )
                else:
                    em_t = wpool.tile([pk, L], F32, tag=f"em_s{li}", bufs=1,
                                      name=f"em{li}")
                    (nc.gpsimd if li % 2 else nc.sync).dma_start(
                        em_t, em_d.ap()[em_off:em_off + n, :])
                em_off += n

                nxt = spool.tile([pk, tk * L], F32, tag=f"lvl{li + 1}",
                                 name=f"lvl{li + 1}")

                # ---- per-level prep: max, subtract, transpose, one copy ----
                mxl = wpool.tile([m0, pchunks], F32, tag="mx", bufs=2,
                                 name="mxl")
                sTp = ppool.tile([L, n_prev], BF16, tag="sTp", bufs=1,
                                 name="sTp")
                for i in range(pchunks):
                    m = m0
                    prev_ap = cur[:m, i * L:(i + 1) * L]
                    nc.vector.reduce_max(mxl[:m, i:i + 1], prev_ap,
                                         axis=mybir.AxisListType.X)
                    scp = wpool.tile([m, L], BF16, tag="scp", bufs=3,
                                     name="scp")
                    nc.vector.tensor_scalar_sub(scp, prev_ap,
                                                mxl[:m, i:i + 1])
                    nc.tensor.transpose(sTp[:, i * 128:i * 128 + m], scp,
                                        ident_bf[:m, :m])
                sT = wpool.tile([L, n_prev], BF16, tag="sT", bufs=2,
                                name="sT")
                nc.scalar.copy(sT, sTp)

                # pair-sum of per-node maxima: one partition->free DMA
                # dd[p, j] = mxl[2p, j] (j<pchunks) else mxl[2p+1, j-pchunks]
                half0 = m0 // 2
                dd = wpool.tile([half0, 2 * pchunks], F32, tag="mx2",
                                bufs=2, name="dd")
                (nc.gpsimd if li % 2 else nc.sync).dma_start(
                    dd, mxl[:m0, 0:pchunks])
                mxs = wpool.tile([pk, tk], F32, tag="mxs", bufs=2,
                                 name="mxs")
                nc.vector.tensor_add(mxs[0:half0, 0:tk],
                                     dd[:, 0:pchunks:2],
                                     dd[:, pchunks:2 * pchunks:2])
                if pchunks > 1:
                    nc.vector.tensor_add(mxs[64:128, 0:tk],
                                         dd[:, 1:pchunks:2],
                                         dd[:, pchunks + 1:2 * pchunks:2])

                # ---- per work tile: B, exp, contraction, ln, out ----
                f = pk                        # nodes per work tile
                for ot in range(max(1, n // f)):
                    ngrp = (f + 127) // 128
                    stacked = wpool.tile([2 * L, f], BF16, tag="stacked",
                                         bufs=3, name="stacked")
                    nc.scalar.copy(stacked[0:L, :],
                                   sT[:, 2 * ot * f:2 * ot * f + 2 * f:2])
                    nc.scalar.copy(stacked[L:2 * L, :],
                                   sT[:, 2 * ot * f + 1:
                                      2 * ot * f + 2 * f:2])

                    bp = ppool.tile([128, 8 * f], F32, tag="bp", bufs=2,
                                    name="bp")
                    for c in range(8):
                        nc.tensor.matmul(bp[:, c * f:(c + 1) * f],
                                         wmat[:, c * 128:(c + 1) * 128],
                                         stacked, start=True, stop=True)
                    outer = wpool.tile([128, 8 * f], BF16, tag="outer",
                                       bufs=2, name="outer")
                    nc.scalar.activation(outer, bp,
                                         mybir.ActivationFunctionType.Exp)

                    ln_s = wpool.tile([L, f], F32, tag="ln_s", bufs=2,
                                      name="ln_s")
                    if n >= 128:
                        st = ppool.tile([L, f], F32, tag="st", bufs=1,
                                        name="st")
                        for c in range(8):
                            nc.tensor.matmul(st, texp[:, c * L:(c + 1) * L],
                                             outer[:, c * f:(c + 1) * f],
                                             start=(c == 0), stop=(c == 7))
                        nc.scalar.activation(ln_s, st,
                                             mybir.ActivationFunctionType.Ln)
                    else:
                        stv = ppool.tile([L, 8 * f], F32, tag="st", bufs=1,
                                         name="stv")
                        for c in range(8):
                            nc.tensor.matmul(stv[:, c * f:(c + 1) * f],
                                             texp[:, c * L:(c + 1) * L],
                                             outer[:, c * f:(c + 1) * f],
                                             start=True, stop=True)
                        ssum = wpool.tile([L, f], F32, tag="ssum", bufs=2,
                                          name="ssum")
                        nc.vector.reduce_sum(
                            ssum, stv.rearrange("p (c f) -> p f c", c=8),
                            axis=mybir.AxisListType.X)
                        nc.scalar.activation(ln_s, ssum,
                                             mybir.ActivationFunctionType.Ln)

                    for g in range(ngrp):
                        gf = min(128, f - g * 128)
                        gc = ot * ngrp + g
                        bt = ppool.tile([gf, L], F32, tag="bt", bufs=2,
                                        name="bt")
                        nc.tensor.transpose(
                            bt, ln_s[:, g * 128:g * 128 + gf], ident[:L, :L])
                        # inside = ln(S) + (mxl+mxr) + em
                        nc.vector.scalar_tensor_tensor(
                            nxt[:, gc * L:(gc + 1) * L], bt,
                            mxs[:gf, gc:gc + 1],
                            em_t[:, gc * L:(gc + 1) * L],
                            op0=mybir.AluOpType.add, op1=mybir.AluOpType.add)

                cur = nxt

            nc.sync.dma_start(out_d.ap(), cur[0:1, 0:L])

    return nc


def _consts():
    trans_lrp = None  # filled by caller for kernel input ordering clarity
    wmat = np.zeros((2 * L, 1024), np.float32)
    for c in range(8):
        for m in range(128):
            wmat[4 * c + m // 32, c * 128 + m] = 1.0
            wmat[L + (m % 32), c * 128 + m] = 1.0
    ident = np.eye(128, dtype=np.float32)
    return wmat, ident


_CACHE = {}
LAST_EXEC_NS = {"A": None, "B": None}


def _split_waits_json(raw, max_waits=1):
    """This container's walrus build allows only one sync-wait command per
    instruction; hoist extra waits into single-wait NoOps on the same engine
    (equivalent: the engine blocks on each in turn)."""
    import orjson

    bir = orjson.loads(raw)
    nextid = 900000
    for fn in bir["functions"]:
        for blk in fn["blocks"]:
            newinsts = []
            for ins in blk["instructions"]:
                si = ins.get("sync_info")
                w = (si or {}).get("on_wait") or []
                while len(w) > max_waits:
                    head, w = w[:max_waits], w[max_waits:]
                    newinsts.append({
                        "name": f"I-W{nextid}", "opcode": "NoOp",
                        "engine": ins["engine"], "ins": [], "outs": [],
                        "sync_info": {"on_update": [], "on_wait": head},
                        "debug": ins.get("debug", 0)})
                    nextid += 1
                if si is not None:
                    si["on_wait"] = w
                newinsts.append(ins)
            blk["instructions"] = newinsts
    return orjson.dumps(bir)


def _get_nc(n0, num_devices):
    key = (n0, num_devices)
    if key not in _CACHE:
        nc = build_tree_nc(n0, num_devices)
        patched = _split_waits_json(nc.to_json_bytes())
        nc.to_json_bytes = lambda: patched
        _CACHE[key] = nc
    return _CACHE[key]


def kernel(leaf_emissions, internal_emissions, trans_matrix):
    leaf_emissions = np.asarray(leaf_emissions, np.float32)
    internal_emissions = np.asarray(internal_emissions, np.float32)
    trans_matrix = np.asarray(trans_matrix, np.float32)

    wmat, ident = _consts()
    trans_lrp = np.ascontiguousarray(
        trans_matrix.transpose(1, 2, 0).reshape(L * L, L))

    # ---- kernel A: 8 subtrees ----
    # per-core emissions: concat of per-level contiguous slices
    g_sizes = _level_sizes(N_LEAVES)          # global level sizes 4096..1
    g_offs = np.concatenate([[0], np.cumsum(g_sizes)])
    sub_levels = 10                           # subtree levels per core
    in_maps = []
    for i in range(N_CORES):
        em_parts = []
        for k in range(sub_levels):
            m = g_sizes[k] // N_CORES
            off = g_offs[k] + i * m
            em_parts.append(internal_emissions[off:off + m])
        in_maps.append({
            "leaves": np.ascontiguousarray(
                leaf_emissions[i * LPC:(i + 1) * LPC]),
            "emissions": np.ascontiguousarray(np.concatenate(em_parts, 0)),
            "trans_lrp": trans_lrp,
            "wmat": wmat,
            "ident": ident,
        })

    trace = bool(os.environ.get("BTC_TRACE"))
    nc_a = _get_nc(LPC, N_CORES)
    res_a = bass_utils.run_bass_kernel_spmd(nc_a, in_maps,
                                            core_ids=list(range(N_CORES)),
                                            trace=trace)
    roots = np.concatenate([res_a.results[i]["root_out"]
                            for i in range(N_CORES)], 0)  # [8, 32]

    # ---- kernel B: top 3 levels ----
    em_top = np.ascontiguousarray(internal_emissions[g_offs[sub_levels]:])
    nc_b = _get_nc(N_CORES, 1)
    res_b = bass_utils.run_bass_kernel_spmd(
        nc_b,
        [{"leaves": np.ascontiguousarray(roots), "emissions": em_top,
          "trans_lrp": trans_lrp, "wmat": wmat, "ident": ident}],
        core_ids=[0], trace=trace)
    LAST_EXEC_NS["A"] = res_a.exec_time_ns
    LAST_EXEC_NS["B"] = res_b.exec_time_ns
    return res_b.results[0]["root_out"].reshape(L)
